# revision 22
# baseline (speedup 1.0000x reference)
"""Batched int8-valued GEMM with dequant epilogue on 8 Trainium2 NeuronCores.

Problem: a[64,1024,128] i32 (vals 0..126), b[64,1024,128] i32 (vals 0..126),
alpha[1] f32.  out[bt,m,n] = fp16(alpha * sum_k a[bt,m,k]*b[bt,n,k]).

Sharding: pure batch-parallel — 8 batches per core, no communication.

Per-core pipeline (per batch), all exact (values <=126 exact in bf16; K=128
-> acc <= 2.03e6 < 2^24 so fp32 PSUM accumulation is exact):
  1. SWDGE cast-DMA loads a/b int32 -> bf16 SBUF [128, 1024] with row
     interleave m = 8p+t so每 partition reads 4 KiB contiguous from HBM.
  2. PE transposes (x identity, groups of 4 into one bf16 PSUM bank) get K
     onto partitions; one DVE/ACT copy per group -> aT/bT bf16.
  3. Matmul pairs [128k x 128m] . [128k x 512n] -> one [128,1024] f32 PSUM
     tile (2 banks), interleaved with the transposes of the next tiles so
     the PE HAM clock gate stays at 2.4 GHz.
  4. Epilogue (DVE/ACT alternating): out = fp16(psum * alpha) in a single
     [128,1024] op with a permuted read AP that undoes the b row interleave.
  5. One 2 MiB HWDGE store per batch (16 KiB contiguous per partition).
"""

import numpy as np

B, M, N, K = 64, 1024, 1024, 128
NCORES = 8
BPC = B // NCORES  # batches per core
TM = M // 128  # m tiles per batch (8)
TN = N // 128  # n tiles per batch (8)

_CACHE = {}


def _build_module():
    from contextlib import ExitStack

    import concourse.tile as tile
    from concourse import bacc, mybir
    from concourse.bass import ds, ts
    from concourse.masks import make_identity

    fp16 = mybir.dt.float16
    bf16 = mybir.dt.bfloat16
    f32 = mybir.dt.float32
    i32 = mybir.dt.int32

    nc = bacc.Bacc("TRN2", debug=False, enable_asserts=False)
    a_d = nc.dram_tensor("a", [BPC, M, K], i32, kind="ExternalInput")
    b_d = nc.dram_tensor("b", [BPC, N, K], i32, kind="ExternalInput")
    al_d = nc.dram_tensor("alpha", [1], f32, kind="ExternalInput")
    o_d = nc.dram_tensor("out", [BPC, M, N], fp16, kind="ExternalOutput")

    with ExitStack() as ctx:
        tc = ctx.enter_context(tile.TileContext(nc))
        const = ctx.enter_context(tc.tile_pool(name="const", bufs=1))
        inp = ctx.enter_context(tc.tile_pool(name="inp", bufs=3))
        io = ctx.enter_context(tc.tile_pool(name="io", bufs=3))
        outp = ctx.enter_context(tc.tile_pool(name="outp", bufs=3))
        pst = ctx.enter_context(tc.tile_pool(name="pst", bufs=2, space="PSUM"))
        psm = ctx.enter_context(tc.tile_pool(name="psm", bufs=3, space="PSUM"))

        # Prefetch the first two batches' inputs before any other gpsimd work
        # so SWDGE emission (and HBM reads) start immediately. b before a —
        # the b transposes are the first consumers.
        in_tiles = {}

        def load_batch(ib, split_b=False):
            b_bf = inp.tile([128, N * K // 128], bf16, tag="b_bf")
            b_src = b_d.ap()[ib].rearrange("(p t) k -> p t k", t=TN)
            b_dst = b_bf[:].rearrange("p (t k) -> p t k", k=K)
            if split_b:
                # two half loads so the first transpose group starts sooner
                nc.gpsimd.dma_start(b_dst[:, 0:4, :], b_src[:, 0:4, :])
                nc.gpsimd.dma_start(b_dst[:, 4:8, :], b_src[:, 4:8, :])
            else:
                nc.gpsimd.dma_start(b_dst, b_src)
            a_bf = inp.tile([128, M * K // 128], bf16, tag="a_bf")
            nc.gpsimd.dma_start(
                a_bf[:].rearrange("p (t k) -> p t k", k=K),
                a_d.ap()[ib].rearrange("(p t) k -> p t k", t=TM),
            )
            in_tiles[ib] = (a_bf, b_bf)

        load_batch(0, split_b=True)
        load_batch(1)

        ident = const.tile([128, 128], bf16)
        make_identity(nc, ident)
        alpha_1 = const.tile([1, 1], f32)
        nc.sync.dma_start(alpha_1[:], al_d.ap().rearrange("(a x) -> a x", a=1))
        ones_row = const.tile([1, 128], f32)
        nc.gpsimd.memset(ones_row[:], 1.0)
        # alpha broadcast to [128,1] via PE: ones_row.T @ alpha (contraction=1)
        alpha_ps = pst.tile([128, 1], f32, tag="ps")
        nc.tensor.matmul(alpha_ps[:], ones_row[:], alpha_1[:], start=True, stop=True)
        alpha_bc = const.tile([128, 1], f32)
        nc.vector.tensor_copy(alpha_bc[:], alpha_ps[:])

        epi_cnt = [0]
        copy_cnt = [0]

        def transpose_group(src_bf, dst_T, g):
            """PE-transpose 4 [128,128] tiles of src into one bf16 psum bank,
            then one copy into dst_T[:, g*512:(g+1)*512]."""
            ps = pst.tile([128, 512], bf16, tag="ps")
            for q in range(4):
                t = 4 * g + q
                nc.tensor.transpose(
                    ps[:, ts(q, 128)], src_bf[:, ts(t, 128)], ident[:]
                )
            if copy_cnt[0] % 2 == 0:
                nc.vector.tensor_copy(dst_T[:, ds(g * 512, 512)], ps[:])
            else:
                nc.scalar.copy(dst_T[:, ds(g * 512, 512)], ps[:])
            copy_cnt[0] += 1

        for ib in range(BPC):
            if ib + 2 < BPC:
                load_batch(ib + 2)
            a_bf, b_bf = in_tiles.pop(ib)

            # aT/bT: [k, j] with j = t*128 + p  <->  row index 8p + t
            aT = io.tile([128, M], bf16, tag="aT")
            bT = io.tile([128, N], bf16, tag="bT")

            for g in range(TN // 4):
                transpose_group(b_bf, bT, g)

            for half in range(2):
                transpose_group(a_bf, aT, half)
                for qq in range(2):
                    # quarter-batch staging: rows 8p+t, t in [2q, 2q+2)
                    q = 2 * half + qq
                    out_sb = outp.tile(
                        [128, M * N // 128 // 4], fp16, tag=f"out_sb{q % 2}"
                    )
                    for tt in range(2):
                        t = 2 * q + tt
                        ps = psm.tile([128, 1024], f32)
                        for nh in range(2):
                            nc.tensor.matmul(
                                ps[:, ds(nh * 512, 512)],
                                aT[:, ts(t, 128)],
                                bT[:, ds(nh * 512, 512)],
                                start=True,
                                stop=True,
                            )
                        # psum free j = t'*128 + p <-> n = 8p + t'; read in
                        # n order: outer p (stride 1, x128), inner t'
                        # (stride 128, x8)
                        ps_n_order = ps[:].rearrange("p (t q) -> p q t", t=8)
                        o_slice = out_sb[:, ds(tt * N, N)].rearrange(
                            "p (q t) -> p q t", t=8
                        )
                        if epi_cnt[0] % 2 == 0:
                            nc.scalar.activation(
                                o_slice,
                                ps_n_order,
                                mybir.ActivationFunctionType.Copy,
                                scale=alpha_bc[:],
                            )
                        else:
                            nc.vector.tensor_scalar_mul(
                                o_slice, ps_n_order, alpha_bc[:]
                            )
                        epi_cnt[0] += 1

                    # rows 8p + (2q + tt): 4 KiB contiguous per partition
                    nc.sync.dma_start(
                        o_d.ap()[ib].rearrange("(p t) n -> p t n", t=TM)[
                            :, 2 * q : 2 * q + 2, :
                        ],
                        out_sb[:].rearrange("p (t n) -> p t n", n=N),
                    )


    nc.compile()
    return nc


def _get_module():
    if "nc" not in _CACHE:
        _CACHE["nc"] = _build_module()
    return _CACHE["nc"]


def run(a, b, alpha, trace=False, **kw):
    from concourse.bass_utils import run_bass_kernel_spmd

    nc = _get_module()
    a = np.ascontiguousarray(a, dtype=np.int32)
    b = np.ascontiguousarray(b, dtype=np.int32)
    alpha = np.ascontiguousarray(alpha, dtype=np.float32)
    in_maps = [
        {
            "a": np.ascontiguousarray(a[i * BPC : (i + 1) * BPC]),
            "b": np.ascontiguousarray(b[i * BPC : (i + 1) * BPC]),
            "alpha": alpha,
        }
        for i in range(NCORES)
    ]
    res = run_bass_kernel_spmd(
        nc, in_maps, core_ids=list(range(NCORES)), trace=trace, **kw
    )
    out = np.concatenate([r["out"] for r in res.results], axis=0)
    return out, res


def kernel(a, b, alpha):
    out, _ = run(a, b, alpha, trace=False)
    return out
